# revision 4
# baseline (speedup 1.0000x reference)
"""Trainium2 Bass kernel for nn_DynamicReindexingRAG (B=4, N=1024, L=128, D=128, Q=64).

Math notes (why this is equivalent to the reference):
- The argsort+gather permutes documents and keys by the SAME permutation; the
  subsequent softmax-weighted sum over all m=N*L positions is permutation
  invariant, so no sorting/gathering is needed.
- K = docs @ Wk.T + bk never needs materializing:
      s[b,m] = qp_sum . K[b,m] = (Wk.T qp_sum) . docs[b,m] + qp_sum.bk
  and the additive constant cancels in softmax.  With qp_sum = Wq qsum + Q bq
  and qsum = Q*qvec, everything folds into  c_t = A2 @ qvec_t + b2  with
      A2 = Q * Wk.T @ Wq,   b2 = Q * Wk.T @ bq.
- Per step:  s = docs_flat @ c   (B, N*L);  w = softmax(s);
  out = w @ docs_flat;  qvec' = 0.5*(qvec + out).
- Sharding: m-dimension split across 8 cores; per-step cross-core softmax
  combine via one AllGather of (local max M_b, local sum S_b, local out O_b).
"""

import numpy as np

B, N, L, D, Q = 4, 1024, 128, 128, 64
NL = N * L
N_CORES = 8
MC = NL // N_CORES          # rows per core per batch (16384)
CHUNK = 2048                # m rows per DMA chunk

_cache = {}


def build(max_steps: int, mc: int = MC, chunk: int = CHUNK,
          n_cores: int = N_CORES):
    import concourse.bass as bass
    import concourse.bacc as bacc
    import concourse.tile as tile
    import concourse.mybir as mybir
    from contextlib import ExitStack

    F32 = mybir.dt.float32
    AF = mybir.ActivationFunctionType
    ALU = mybir.AluOpType
    AX = mybir.AxisListType

    nchunk = mc // chunk
    tpc = chunk // 128          # 128-row tiles per chunk
    ntile = mc // 128
    pay = 8 + B * D             # payload floats: [M(4) | S(4) | O(4*128)]

    nc = bacc.Bacc("TRN2", target_bir_lowering=False, debug=False,
                   num_devices=n_cores)
    docs_ap = nc.dram_tensor("docs", [B, mc, D], F32, kind="ExternalInput").ap()
    a2t_ap = nc.dram_tensor("a2t", [D, D], F32, kind="ExternalInput").ap()
    b2x_ap = nc.dram_tensor("b2x", [D, B], F32, kind="ExternalInput").ap()
    qv0_ap = nc.dram_tensor("qv0", [D, B], F32, kind="ExternalInput").ap()
    ident_ap = nc.dram_tensor("ident", [128, 128], F32, kind="ExternalInput").ap()
    outs_ap = nc.dram_tensor("outs", [max_steps * B, D], F32,
                             kind="ExternalOutput").ap()

    with tile.TileContext(nc) as tc, ExitStack() as ctx:
        const = ctx.enter_context(tc.tile_pool(name="const", bufs=1))
        state = ctx.enter_context(tc.tile_pool(name="state", bufs=1))
        chunks1 = ctx.enter_context(tc.tile_pool(name="chunks1", bufs=4))
        chunks2 = ctx.enter_context(tc.tile_pool(name="chunks2", bufs=4))
        prods = ctx.enter_context(tc.tile_pool(name="prods", bufs=2))
        work = ctx.enter_context(tc.tile_pool(name="work", bufs=3))
        small = ctx.enter_context(tc.tile_pool(name="small", bufs=6))
        # PSUM: 8 banks total.  2 (big) + 2 (o) + 3 (small shared) = 7.
        ps_big = ctx.enter_context(tc.tile_pool(name="ps_big", bufs=2, space="PSUM"))
        ps_o = ctx.enter_context(tc.tile_pool(name="ps_o", bufs=2, space="PSUM"))
        ps_sm = ctx.enter_context(tc.tile_pool(name="ps_sm", bufs=3, space="PSUM"))
        dram = ctx.enter_context(tc.tile_pool(name="dram", bufs=1, space="DRAM"))

        # ---- constants / state ----
        a2t = const.tile([D, D], F32)
        nc.sync.dma_start(a2t[:], a2t_ap[:])
        b2x = const.tile([D, B], F32)
        nc.sync.dma_start(b2x[:], b2x_ap[:])
        ident = const.tile([128, 128], F32)
        nc.sync.dma_start(ident[:], ident_ap[:])
        ones_row = const.tile([1, 128], F32)
        nc.vector.memset(ones_row[:], 1.0)
        ones_col = const.tile([128, 1], F32)
        nc.vector.memset(ones_col[:], 1.0)

        qv = state.tile([D, B], F32)          # current qvec per batch (columns)
        nc.sync.dma_start(qv[:], qv0_ap[:])
        results = state.tile([D, max_steps * B], F32)

        cc_in = dram.tile([1, pay], F32)
        cc_out = dram.tile([n_cores, pay], F32)

        for t in range(max_steps):
            # ---- c_t = A2 @ qvec + b2  (all batches at once) ----
            c_ps = ps_sm.tile([D, B], F32, tag="sm")
            nc.tensor.matmul(c_ps[:], a2t[:], qv[:], start=True, stop=True)
            c_sb = work.tile([D, B], F32, tag="c_sb")
            nc.vector.tensor_tensor(c_sb[:], c_ps[:], b2x[:], op=ALU.add)
            payload = work.tile([1, pay], F32, tag="payload")

            for b in range(B):
                # c_b as a row [1, D] on partition 0
                cT_ps = ps_sm.tile([1, D], F32, tag="sm")
                nc.tensor.transpose(cT_ps[:], c_sb[:, b:b + 1], ident[:])
                cT = work.tile([1, D], F32, tag="cT")
                nc.scalar.copy(cT[:], cT_ps[:])
                # c broadcast to all partitions: c_bc[p, d] = c[d, b]
                c_bc_ps = ps_big.tile([128, D], F32, tag="big")
                nc.tensor.matmul(c_bc_ps[:], ones_row[:], cT[:],
                                 start=True, stop=True)
                c_bc = work.tile([128, D], F32, tag="c_bc")
                nc.scalar.copy(c_bc[:], c_bc_ps[:])
                c_bc3 = c_bc[:].rearrange("p (o d) -> p o d", o=1)

                # ---- pass 1: s = docs . c ----
                s_buf = work.tile([128, ntile], F32, tag="s_buf")
                for k in range(nchunk):
                    ch = chunks1.tile([128, chunk], F32, tag="ch1")
                    src = docs_ap[b, k * chunk:(k + 1) * chunk, :]
                    nc.sync.dma_start(
                        ch[:].rearrange("p (t d) -> p t d", t=tpc),
                        src.rearrange("(t p) d -> p t d", p=128))
                    prod = prods.tile([128, chunk], F32, tag="prod")
                    ch3 = ch[:].rearrange("p (t d) -> p t d", t=tpc)
                    pr3 = prod[:].rearrange("p (t d) -> p t d", t=tpc)
                    i0, i1 = bass.broadcast_tensor_aps(ch3, c_bc3)
                    nc.vector.tensor_tensor(pr3, i0, i1, op=ALU.mult)
                    nc.vector.tensor_reduce(
                        s_buf[:, k * tpc:(k + 1) * tpc], pr3, axis=AX.X,
                        op=ALU.add)

                # ---- local max ----
                m1 = small.tile([128, 1], F32, tag="m1")
                nc.vector.tensor_reduce(m1[:], s_buf[:], axis=AX.X, op=ALU.max)
                mT_ps = ps_sm.tile([1, 128], F32, tag="sm")
                nc.tensor.transpose(mT_ps[:], m1[:], ident[:])
                mrow = small.tile([1, 128], F32, tag="mrow")
                nc.scalar.copy(mrow[:], mT_ps[:])
                Mloc = small.tile([1, 1], F32, tag="Mloc")
                nc.vector.tensor_reduce(Mloc[:], mrow[:], axis=AX.X, op=ALU.max)
                nc.vector.tensor_copy(payload[0:1, b:b + 1], Mloc[:])
                negM = small.tile([1, 1], F32, tag="negM")
                nc.scalar.mul(negM[:], Mloc[:], -1.0)
                negM_ps = ps_sm.tile([128, 1], F32, tag="sm")
                nc.tensor.matmul(negM_ps[:], ones_row[:], negM[:],
                                 start=True, stop=True)
                negM_bc = small.tile([128, 1], F32, tag="negM_bc")
                nc.vector.tensor_copy(negM_bc[:], negM_ps[:])

                # ---- w = exp(s - M), wsum per partition ----
                w_buf = work.tile([128, ntile], F32, tag="w_buf")
                wsum = small.tile([128, 1], F32, tag="wsum")
                nc.scalar.activation(w_buf[:], s_buf[:], AF.Exp,
                                     bias=negM_bc[:], scale=1.0,
                                     accum_out=wsum[:])
                # local S = sum over partitions of wsum
                S_ps = ps_sm.tile([1, 1], F32, tag="sm")
                nc.tensor.matmul(S_ps[:], wsum[:], ones_col[:],
                                 start=True, stop=True)
                nc.scalar.copy(payload[0:1, B + b:B + b + 1], S_ps[:])

                # ---- pass 2: O = sum_m w_m * docs_m  (row [1, D]) ----
                o_ps = ps_o.tile([1, D], F32, tag="o")
                for k in range(nchunk):
                    ch = chunks2.tile([128, chunk], F32, tag="ch2")
                    src = docs_ap[b, k * chunk:(k + 1) * chunk, :]
                    nc.sync.dma_start(
                        ch[:].rearrange("p (t d) -> p t d", t=tpc),
                        src.rearrange("(t p) d -> p t d", p=128))
                    for j in range(tpc):
                        idx = k * tpc + j
                        nc.tensor.matmul(
                            o_ps[:], w_buf[:, idx:idx + 1],
                            ch[:, j * 128:(j + 1) * 128],
                            start=(idx == 0), stop=(idx == ntile - 1))
                nc.scalar.copy(payload[0:1, 8 + b * D:8 + (b + 1) * D], o_ps[:])

            # ---- cross-core combine ----
            nc.sync.dma_start(cc_in[:], payload[:])
            nc.gpsimd.collective_compute(
                "AllGather", mybir.AluOpType.bypass,
                replica_groups=[list(range(n_cores))],
                ins=[cc_in.opt()], outs=[cc_out.opt()])
            gath = work.tile([n_cores, pay], F32, tag="gath")
            nc.sync.dma_start(gath[:], cc_out[:])

            # global max per batch: gath[:, 0:4] -> [4, n_cores] -> max
            gmT_ps = ps_sm.tile([B, n_cores], F32, tag="sm")
            nc.tensor.transpose(gmT_ps[:], gath[:, 0:B], ident[0:n_cores, 0:n_cores])
            gmT = small.tile([B, n_cores], F32, tag="gmT")
            nc.scalar.copy(gmT[:], gmT_ps[:])
            Mg = small.tile([B, 1], F32, tag="Mg")
            nc.vector.tensor_reduce(Mg[:], gmT[:], axis=AX.X, op=ALU.max)
            MgT_ps = ps_sm.tile([1, B], F32, tag="sm")
            nc.tensor.transpose(MgT_ps[:], Mg[:], ident[0:B, 0:B])
            negMgT = small.tile([1, B], F32, tag="negMgT")
            nc.scalar.mul(negMgT[:], MgT_ps[:], -1.0)
            negMg_ps = ps_sm.tile([n_cores, B], F32, tag="sm")
            nc.tensor.matmul(negMg_ps[:], ones_row[0:1, 0:n_cores], negMgT[:],
                             start=True, stop=True)
            shift = small.tile([n_cores, B], F32, tag="shift")
            nc.vector.tensor_tensor(shift[:], gath[:, 0:B], negMg_ps[:], op=ALU.add)
            f_mat = small.tile([n_cores, B], F32, tag="f_mat")
            nc.scalar.activation(f_mat[:], shift[:], AF.Exp)

            for b in range(B):
                St_ps = ps_sm.tile([1, 1], F32, tag="sm")
                nc.tensor.matmul(St_ps[:], gath[:, B + b:B + b + 1],
                                 f_mat[:, b:b + 1], start=True, stop=True)
                St = small.tile([1, 1], F32, tag="St")
                nc.vector.tensor_copy(St[:], St_ps[:])
                rS = small.tile([1, 1], F32, tag="rS")
                nc.vector.reciprocal(rS[:], St[:])
                rS8_ps = ps_sm.tile([n_cores, 1], F32, tag="sm")
                nc.tensor.matmul(rS8_ps[:], ones_row[0:1, 0:n_cores], rS[:],
                                 start=True, stop=True)
                f2 = small.tile([n_cores, 1], F32, tag="f2")
                nc.vector.tensor_tensor(f2[:], f_mat[:, b:b + 1], rS8_ps[:],
                                        op=ALU.mult)
                oc_ps = ps_sm.tile([D, 1], F32, tag="sm")
                nc.tensor.matmul(oc_ps[:], gath[:, 8 + b * D:8 + (b + 1) * D],
                                 f2[:], start=True, stop=True)
                out_col = small.tile([D, 1], F32, tag="out_col")
                nc.vector.tensor_copy(out_col[:], oc_ps[:])
                nc.vector.tensor_copy(results[:, t * B + b:t * B + b + 1],
                                      out_col[:])
                # qvec' = 0.5*(qvec + out)
                nc.vector.tensor_scalar(
                    qv[:, b:b + 1], qv[:, b:b + 1], out_col[:], 0.5,
                    op0=ALU.add, op1=ALU.mult)

        # ---- write outputs: results [D, S*B] -> outs [S*B, D] ----
        n_out = max_steps * B
        res_ps = ps_big.tile([n_out, D], F32, tag="big")
        nc.tensor.transpose(res_ps[:], results[:], ident[:])
        res_T = work.tile([n_out, D], F32, tag="res_T")
        nc.scalar.copy(res_T[:], res_ps[:])
        nc.sync.dma_start(outs_ap[:], res_T[:])

    nc.compile()
    return nc


def make_inputs(query, documents, Wq, bq, Wk, bk,
                mc: int = MC, n_cores: int = N_CORES):
    """Host-side preprocessing -> per-core input maps."""
    query = np.asarray(query, dtype=np.float32)
    documents = np.asarray(documents, dtype=np.float32)
    Wq64 = np.asarray(Wq, dtype=np.float64)
    bq64 = np.asarray(bq, dtype=np.float64)
    Wk64 = np.asarray(Wk, dtype=np.float64)

    A2 = Q * (Wk64.T @ Wq64)
    b2 = Q * (Wk64.T @ bq64)
    a2t = np.ascontiguousarray(A2.T.astype(np.float32))          # [j, i] layout
    b2x = np.ascontiguousarray(
        np.repeat(b2.astype(np.float32)[:, None], B, axis=1))    # [D, B]
    qv0 = np.ascontiguousarray(
        query.astype(np.float64).mean(axis=1).T.astype(np.float32))  # [D, B]
    ident = np.eye(128, dtype=np.float32)

    nl = documents.shape[1] * documents.shape[2]
    dflat = documents.reshape(B, nl, D)
    in_maps = []
    for c in range(n_cores):
        shard = np.ascontiguousarray(dflat[:, c * mc:(c + 1) * mc, :])
        in_maps.append({"docs": shard, "a2t": a2t, "b2x": b2x,
                        "qv0": qv0, "ident": ident})
    return in_maps


def kernel(query, documents, Wq, bq, Wk, bk, max_steps):
    from concourse.bass_utils import run_bass_kernel_spmd

    steps = int(max_steps)
    if steps not in _cache:
        _cache[steps] = build(steps)
    nc = _cache[steps]

    in_maps = make_inputs(query, documents, Wq, bq, Wk, bk)
    res = run_bass_kernel_spmd(nc, in_maps, core_ids=list(range(N_CORES)))
    outs = res.results[0]["outs"]                     # [steps*B, D], t-major
    return np.ascontiguousarray(
        outs.reshape(steps, B, D).transpose(1, 0, 2))  # (B, steps, D)


# revision 8
# speedup vs baseline: 3.0750x; 3.0750x over previous
"""Trainium2 Bass kernel for nn_DynamicReindexingRAG (B=4, N=1024, L=128, D=128, Q=64).

Math notes (why this is equivalent to the reference):
- The argsort+gather permutes documents and keys by the SAME permutation; the
  subsequent softmax-weighted sum over all m=N*L positions is permutation
  invariant, so no sorting/gathering is needed.
- K = docs @ Wk.T + bk never needs materializing:
      s[b,m] = qp_sum . K[b,m] = (Wk.T qp_sum) . docs[b,m] + qp_sum.bk
  and the additive constant cancels in softmax.  With qp_sum = Wq qsum + Q bq
  and qsum = Q*qvec, everything folds into  c_t = A2 @ qvec_t + b2  with
      A2 = Q * Wk.T @ Wq,   b2 = Q * Wk.T @ bq.
- Per step:  s = docs_flat @ c   (B, N*L);  w = softmax(s);
  out = w @ docs_flat;  qvec' = 0.5*(qvec + out).
- Sharding: m-dimension split across 8 cores; per-step cross-core softmax
  combine via one AllGather of (local max M_b, local sum S_b, local out O_b).
"""

import numpy as np

B, N, L, D, Q = 4, 1024, 128, 128, 64
NL = N * L
N_CORES = 8
MC = NL // N_CORES          # rows per core per batch (16384)
CHUNK = 2048                # m rows per DMA chunk

_cache = {}


def build(max_steps: int, mc: int = MC, chunk: int = CHUNK,
          n_cores: int = N_CORES):
    import concourse.bass as bass
    import concourse.bacc as bacc
    import concourse.tile as tile
    import concourse.mybir as mybir
    from contextlib import ExitStack

    F32 = mybir.dt.float32
    AF = mybir.ActivationFunctionType
    ALU = mybir.AluOpType
    AX = mybir.AxisListType

    nchunk = mc // chunk
    tpc = chunk // 128          # 128-row tiles per chunk
    ntile = mc // 128
    pay = 8 + B * D             # payload floats: [M(4) | S(4) | O(4*128)]

    nc = bacc.Bacc("TRN2", target_bir_lowering=False, debug=False,
                   num_devices=n_cores)
    docs_ap = nc.dram_tensor("docs", [B, mc, D], F32, kind="ExternalInput").ap()
    a2t_ap = nc.dram_tensor("a2t", [D, D], F32, kind="ExternalInput").ap()
    b2x_ap = nc.dram_tensor("b2x", [D, B], F32, kind="ExternalInput").ap()
    qv0_ap = nc.dram_tensor("qv0", [D, B], F32, kind="ExternalInput").ap()
    ident_ap = nc.dram_tensor("ident", [128, 128], F32, kind="ExternalInput").ap()
    outs_ap = nc.dram_tensor("outs", [max_steps * B, D], F32,
                             kind="ExternalOutput").ap()

    with tile.TileContext(nc) as tc, ExitStack() as ctx:
        const = ctx.enter_context(tc.tile_pool(name="const", bufs=1))
        state = ctx.enter_context(tc.tile_pool(name="state", bufs=1))
        chunks1 = ctx.enter_context(tc.tile_pool(name="chunks1", bufs=12))
        prods = ctx.enter_context(tc.tile_pool(name="prods", bufs=2))
        work = ctx.enter_context(tc.tile_pool(name="work", bufs=3))
        cbc_pool = ctx.enter_context(tc.tile_pool(name="cbc", bufs=5))
        small = ctx.enter_context(tc.tile_pool(name="small", bufs=6))
        # PSUM: 8 banks total.  2 (big) + 2 (o) + 3 (small shared) = 7.
        ps_big = ctx.enter_context(tc.tile_pool(name="ps_big", bufs=2, space="PSUM"))
        ps_o = ctx.enter_context(tc.tile_pool(name="ps_o", bufs=2, space="PSUM"))
        ps_sm = ctx.enter_context(tc.tile_pool(name="ps_sm", bufs=3, space="PSUM"))
        dram = ctx.enter_context(tc.tile_pool(name="dram", bufs=1, space="DRAM"))

        # ---- constants / state ----
        a2t = const.tile([D, D], F32)
        nc.sync.dma_start(a2t[:], a2t_ap[:])
        b2x = const.tile([D, B], F32)
        nc.sync.dma_start(b2x[:], b2x_ap[:])
        ident = const.tile([128, 128], F32)
        nc.sync.dma_start(ident[:], ident_ap[:])
        ones_row = const.tile([1, 128], F32)
        nc.vector.memset(ones_row[:], 1.0)
        ones_col = const.tile([128, 1], F32)
        nc.vector.memset(ones_col[:], 1.0)

        qv = state.tile([D, B], F32)          # current qvec per batch (columns)
        nc.sync.dma_start(qv[:], qv0_ap[:])
        results = state.tile([D, max_steps * B], F32)

        cc_in = dram.tile([1, pay], F32)
        cc_out = dram.tile([n_cores, pay], F32)

        for t in range(max_steps):
            # ---- c_t = A2 @ qvec + b2  (all batches at once) ----
            c_ps = ps_sm.tile([D, B], F32, tag="sm")
            nc.tensor.matmul(c_ps[:], a2t[:], qv[:], start=True, stop=True)
            c_sb = work.tile([D, B], F32, tag="c_sb")
            nc.vector.tensor_tensor(c_sb[:], c_ps[:], b2x[:], op=ALU.add)
            payload = work.tile([1, pay], F32, tag="payload")

            # build ALL batches' c broadcasts up-front so the PE ops for
            # batch b+1's pass-1 never sit behind batch b's pass-2 matmuls
            # in the in-order PE stream (otherwise DVE/PE fully serialize).
            c_bcs = []
            for b in range(B):
                cT_ps = ps_sm.tile([1, D], F32, tag="sm")
                nc.tensor.transpose(cT_ps[:], c_sb[:, b:b + 1], ident[:])
                cT = work.tile([1, D], F32, tag="cT")
                nc.scalar.copy(cT[:], cT_ps[:])
                c_bc_ps = ps_big.tile([128, D], F32, tag="big")
                nc.tensor.matmul(c_bc_ps[:], ones_row[:], cT[:],
                                 start=True, stop=True)
                c_bc = cbc_pool.tile([128, D], F32, tag="c_bc")
                nc.scalar.copy(c_bc[:], c_bc_ps[:])
                c_bcs.append(c_bc)

            for b in range(B):
                c_bc3 = c_bcs[b][:].rearrange("p (o d) -> p o d", o=1)

                # ---- pass 1: s = docs . c  (chunks stay in SBUF) ----
                s_buf = work.tile([128, ntile], F32, tag="s_buf")
                chs = []
                for k in range(nchunk):
                    ch = chunks1.tile([128, chunk], F32, tag="ch1")
                    chs.append(ch)
                    src = docs_ap[b, k * chunk:(k + 1) * chunk, :]
                    nc.sync.dma_start(
                        ch[:].rearrange("p (t d) -> p t d", t=tpc),
                        src.rearrange("(t p) d -> p t d", p=128))
                    prod = prods.tile([128, chunk], F32, tag="prod")
                    ch3 = ch[:].rearrange("p (t d) -> p t d", t=tpc)
                    pr3 = prod[:].rearrange("p (t d) -> p t d", t=tpc)
                    i0, i1 = bass.broadcast_tensor_aps(ch3, c_bc3)
                    nc.vector.tensor_tensor(pr3, i0, i1, op=ALU.mult)
                    nc.vector.tensor_reduce(
                        s_buf[:, k * tpc:(k + 1) * tpc], pr3, axis=AX.X,
                        op=ALU.add)

                # ---- local max ----
                m1 = small.tile([128, 1], F32, tag="m1")
                nc.vector.tensor_reduce(m1[:], s_buf[:], axis=AX.X, op=ALU.max)
                mT_ps = ps_sm.tile([1, 128], F32, tag="sm")
                nc.tensor.transpose(mT_ps[:], m1[:], ident[:])
                mrow = small.tile([1, 128], F32, tag="mrow")
                nc.scalar.copy(mrow[:], mT_ps[:])
                Mloc = small.tile([1, 1], F32, tag="Mloc")
                nc.vector.tensor_reduce(Mloc[:], mrow[:], axis=AX.X, op=ALU.max)
                nc.vector.tensor_copy(payload[0:1, b:b + 1], Mloc[:])
                negM = small.tile([1, 1], F32, tag="negM")
                nc.scalar.mul(negM[:], Mloc[:], -1.0)
                negM_ps = ps_sm.tile([128, 1], F32, tag="sm")
                nc.tensor.matmul(negM_ps[:], ones_row[:], negM[:],
                                 start=True, stop=True)
                negM_bc = small.tile([128, 1], F32, tag="negM_bc")
                nc.vector.tensor_copy(negM_bc[:], negM_ps[:])

                # ---- w = exp(s - M), wsum per partition ----
                w_buf = work.tile([128, ntile], F32, tag="w_buf")
                wsum = small.tile([128, 1], F32, tag="wsum")
                nc.scalar.activation(w_buf[:], s_buf[:], AF.Exp,
                                     bias=negM_bc[:], scale=1.0,
                                     accum_out=wsum[:])
                # local S = sum over partitions of wsum
                S_ps = ps_sm.tile([1, 1], F32, tag="sm")
                nc.tensor.matmul(S_ps[:], wsum[:], ones_col[:],
                                 start=True, stop=True)
                nc.scalar.copy(payload[0:1, B + b:B + b + 1], S_ps[:])

                # ---- pass 2: O = sum_m w_m * docs_m, from SBUF-resident chunks ----
                o_ps = ps_o.tile([1, D], F32, tag="o")
                for k in range(nchunk):
                    ch = chs[k]
                    for j in range(tpc):
                        idx = k * tpc + j
                        nc.tensor.matmul(
                            o_ps[:], w_buf[:, idx:idx + 1],
                            ch[:, j * 128:(j + 1) * 128],
                            start=(idx == 0), stop=(idx == ntile - 1))
                nc.scalar.copy(payload[0:1, 8 + b * D:8 + (b + 1) * D], o_ps[:])

            # ---- cross-core combine ----
            nc.sync.dma_start(cc_in[:], payload[:])
            nc.gpsimd.collective_compute(
                "AllGather", mybir.AluOpType.bypass,
                replica_groups=[list(range(n_cores))],
                ins=[cc_in.opt()], outs=[cc_out.opt()])
            gath = work.tile([n_cores, pay], F32, tag="gath")
            nc.sync.dma_start(gath[:], cc_out[:])

            # global max per batch: gath[:, 0:4] -> [4, n_cores] -> max
            gmT_ps = ps_sm.tile([B, n_cores], F32, tag="sm")
            nc.tensor.transpose(gmT_ps[:], gath[:, 0:B], ident[0:n_cores, 0:n_cores])
            gmT = small.tile([B, n_cores], F32, tag="gmT")
            nc.scalar.copy(gmT[:], gmT_ps[:])
            Mg = small.tile([B, 1], F32, tag="Mg")
            nc.vector.tensor_reduce(Mg[:], gmT[:], axis=AX.X, op=ALU.max)
            MgT_ps = ps_sm.tile([1, B], F32, tag="sm")
            nc.tensor.transpose(MgT_ps[:], Mg[:], ident[0:B, 0:B])
            negMgT = small.tile([1, B], F32, tag="negMgT")
            nc.scalar.mul(negMgT[:], MgT_ps[:], -1.0)
            negMg_ps = ps_sm.tile([n_cores, B], F32, tag="sm")
            nc.tensor.matmul(negMg_ps[:], ones_row[0:1, 0:n_cores], negMgT[:],
                             start=True, stop=True)
            shift = small.tile([n_cores, B], F32, tag="shift")
            nc.vector.tensor_tensor(shift[:], gath[:, 0:B], negMg_ps[:], op=ALU.add)
            f_mat = small.tile([n_cores, B], F32, tag="f_mat")
            nc.scalar.activation(f_mat[:], shift[:], AF.Exp)

            for b in range(B):
                St_ps = ps_sm.tile([1, 1], F32, tag="sm")
                nc.tensor.matmul(St_ps[:], gath[:, B + b:B + b + 1],
                                 f_mat[:, b:b + 1], start=True, stop=True)
                St = small.tile([1, 1], F32, tag="St")
                nc.vector.tensor_copy(St[:], St_ps[:])
                rS = small.tile([1, 1], F32, tag="rS")
                nc.vector.reciprocal(rS[:], St[:])
                rS8_ps = ps_sm.tile([n_cores, 1], F32, tag="sm")
                nc.tensor.matmul(rS8_ps[:], ones_row[0:1, 0:n_cores], rS[:],
                                 start=True, stop=True)
                f2 = small.tile([n_cores, 1], F32, tag="f2")
                nc.vector.tensor_tensor(f2[:], f_mat[:, b:b + 1], rS8_ps[:],
                                        op=ALU.mult)
                oc_ps = ps_sm.tile([D, 1], F32, tag="sm")
                nc.tensor.matmul(oc_ps[:], gath[:, 8 + b * D:8 + (b + 1) * D],
                                 f2[:], start=True, stop=True)
                out_col = small.tile([D, 1], F32, tag="out_col")
                nc.vector.tensor_copy(out_col[:], oc_ps[:])
                nc.vector.tensor_copy(results[:, t * B + b:t * B + b + 1],
                                      out_col[:])
                # qvec' = 0.5*(qvec + out)
                nc.vector.tensor_scalar(
                    qv[:, b:b + 1], qv[:, b:b + 1], out_col[:], 0.5,
                    op0=ALU.add, op1=ALU.mult)

        # ---- write outputs: results [D, S*B] -> outs [S*B, D] ----
        n_out = max_steps * B
        res_ps = ps_big.tile([n_out, D], F32, tag="big")
        nc.tensor.transpose(res_ps[:], results[:], ident[:])
        res_T = work.tile([n_out, D], F32, tag="res_T")
        nc.scalar.copy(res_T[:], res_ps[:])
        nc.sync.dma_start(outs_ap[:], res_T[:])

    nc.compile()
    return nc


def make_inputs(query, documents, Wq, bq, Wk, bk,
                mc: int = MC, n_cores: int = N_CORES):
    """Host-side preprocessing -> per-core input maps."""
    query = np.asarray(query, dtype=np.float32)
    documents = np.asarray(documents, dtype=np.float32)
    Wq64 = np.asarray(Wq, dtype=np.float64)
    bq64 = np.asarray(bq, dtype=np.float64)
    Wk64 = np.asarray(Wk, dtype=np.float64)

    A2 = Q * (Wk64.T @ Wq64)
    b2 = Q * (Wk64.T @ bq64)
    a2t = np.ascontiguousarray(A2.T.astype(np.float32))          # [j, i] layout
    b2x = np.ascontiguousarray(
        np.repeat(b2.astype(np.float32)[:, None], B, axis=1))    # [D, B]
    qv0 = np.ascontiguousarray(
        query.astype(np.float64).mean(axis=1).T.astype(np.float32))  # [D, B]
    ident = np.eye(128, dtype=np.float32)

    nl = documents.shape[1] * documents.shape[2]
    dflat = documents.reshape(B, nl, D)
    in_maps = []
    for c in range(n_cores):
        shard = np.ascontiguousarray(dflat[:, c * mc:(c + 1) * mc, :])
        in_maps.append({"docs": shard, "a2t": a2t, "b2x": b2x,
                        "qv0": qv0, "ident": ident})
    return in_maps


def kernel(query, documents, Wq, bq, Wk, bk, max_steps):
    from concourse.bass_utils import run_bass_kernel_spmd

    steps = int(max_steps)
    if steps not in _cache:
        _cache[steps] = build(steps)
    nc = _cache[steps]

    in_maps = make_inputs(query, documents, Wq, bq, Wk, bk)
    res = run_bass_kernel_spmd(nc, in_maps, core_ids=list(range(N_CORES)))
    outs = res.results[0]["outs"]                     # [steps*B, D], t-major
    return np.ascontiguousarray(
        outs.reshape(steps, B, D).transpose(1, 0, 2))  # (B, steps, D)
